# revision 1
# baseline (speedup 1.0000x reference)
"""DeepCoevolve on Trainium2 (Bass/Tile), 8 NeuronCores.

Strategy
--------
The event scan is sequential only through rows that are touched more than
once.  With 4096 random events over 100k users / 50k items the dependency
DAG is shallow (~5 wavefront levels) and splits into ~3900 tiny connected
components.  So:

  host:   . wavefront-level each event  (level = 1 + max(level of prev event
            sharing its user or item))
          . union-find connected components, pack them onto 8 cores
            (zero cross-core dependencies)
          . rename scatter targets: event #e writes its GRU outputs to its
            own private column pair, so the device never scatters -- each
            step writes one contiguous column block and only the *gather*
            is indirect (precomputed int16 indices, ap_gather on GPSIMD)
          . pre-gather every event input that comes from the *initial*
            tables (94% of events are wavefront-0) into the HS staging
            buffer on the host; the device only gathers columns that chain
            to an earlier event's GRU output (~4% of slots), reordered to
            the front of each step so one contiguous prefix gather suffices
  device: . one unified SBUF value buffer VBUF [128, cols]:
              [user init rows | item init rows | per-step output blocks]
          . per step (wavefront chunk, B events, all independent):
              prefix ap_gather of chained u / v columns (none for level 0)
              + fp32r rounding CAST of the gathered prefix
              16 fp32r matmuls -> 4 PSUM gate tiles [128, 2B]
                (biases folded in via K=2 matmuls against a 0/1 selector)
              3 ACT + 5 DVE elementwise ops at double width (user cell in
              cols [0,B), item cell in [B,2B)) -> write block into VBUF
          . MLP scores + softplus losses for all events in step-aligned
            ~500-wide batched passes (the big level-0 chunk has no device
            dependencies, so it overlaps the GRU step loop)
  output: [1, ne] loss + [1, ne] score per core; host reassembles [4096, 2]
          (negating the log term on the host).

fp32r notes: matmul operands must be *produced* as float32r (11-bit
mantissa).  Host-shipped operands are pre-rounded and DMA'd as f32r;
gathered columns pass through a DVE CAST; ap_gather itself only supports
plain dtypes.  The gather ucode also reads its int16 index array in 32-bit
pairs, so every step's index block starts on an even 16-index column.
"""

import numpy as np
from contextlib import ExitStack

E = 128
NCORES = 8
LANE = 16        # ap_gather index granularity
MAXB = 256       # max events per step (2B <= 512 f32 = one PSUM bank)

_CACHE = {}
LAST_EXEC_NS = None
TRACE = False


def _round16(x):
    return max(LANE, (int(x) + LANE - 1) // LANE * LANE)


def _round_fp32r(x):
    """Round fp32 -> fp32r bit format (11-bit mantissa, low 12 bits zero)."""
    b = np.ascontiguousarray(x, np.float32).view(np.uint32)
    lsb = (b >> 12) & 1
    return ((b + 0x7FF + lsb) & 0xFFFF_F000).view(np.float32)


class _Schedule:
    pass


# ----------------------------------------------------------------------------
# host-side scheduling
# ----------------------------------------------------------------------------

def _build_schedule(uid, iid):
    """Wavefront + component schedule. Pure numpy/python, deterministic."""
    uid = np.asarray(uid, np.int64)
    iid = np.asarray(iid, np.int64)
    nev = len(uid)

    # --- wavefront levels ---------------------------------------------------
    lvl = np.zeros(nev, np.int32)
    last_u, last_i = {}, {}
    parent = list(range(nev))

    def find(x):
        while parent[x] != x:
            parent[x] = parent[parent[x]]
            x = parent[x]
        return x

    def union(a, b):
        ra, rb = find(a), find(b)
        if ra != rb:
            parent[ra] = rb

    for e in range(nev):
        l = 0
        a = last_u.get(uid[e])
        if a is not None:
            l = lvl[a] + 1
            union(e, a)
        b = last_i.get(iid[e])
        if b is not None:
            l = max(l, lvl[b] + 1)
            union(e, b)
        lvl[e] = l
        last_u[uid[e]] = e
        last_i[iid[e]] = e

    nlev = int(lvl.max()) + 1

    # --- components -> cores ------------------------------------------------
    comps = {}
    for e in range(nev):
        comps.setdefault(find(e), []).append(e)
    comp_list = sorted(comps.values(), key=len, reverse=True)
    core_events = [[] for _ in range(NCORES)]
    core_tot = [0] * NCORES
    for c in comp_list:
        k = min(range(NCORES), key=lambda i: core_tot[i])
        core_events[k].extend(c)
        core_tot[k] += len(c)

    # "chained" = this event's u (or v) row was touched by an earlier event.
    # Chained relative to the whole stream == chained within its core,
    # because components are assigned whole.
    chained_u = np.zeros(nev, bool)
    chained_v = np.zeros(nev, bool)
    seen_u, seen_i = set(), set()
    for e in range(nev):
        chained_u[e] = uid[e] in seen_u
        chained_v[e] = iid[e] in seen_i
        seen_u.add(uid[e])
        seen_i.add(iid[e])

    # per-core, per-level event queues; within a level, chained-u events
    # first, then chained-v, then pure-init: each step then needs only a
    # prefix gather on the device.
    queues = [[[] for _ in range(nlev)] for _ in range(NCORES)]
    for k in range(NCORES):
        for e in sorted(core_events[k]):
            queues[k][lvl[e]].append(e)
    for k in range(NCORES):
        for l in range(nlev):
            queues[k][l].sort(
                key=lambda e: (not chained_u[e], not chained_v[e], e))

    # --- step structure (shared by all cores) -------------------------------
    lev_sizes = [_round16(max(len(queues[k][l]) for k in range(NCORES)))
                 for l in range(nlev)]
    steps = []              # [level, B, off, icol]
    off = 0
    icol = 0                # idx-array column start; kept EVEN (ucode reads
    for l, m in enumerate(lev_sizes):       # int16 idx pairs as 32-bit words)
        rem = m
        while rem > 0:
            b = min(MAXB, rem)
            steps.append([l, b, off, icol])
            off += b
            icol += (b // LANE + 1) // 2 * 2
            rem -= b
    ne = off
    nicol = icol

    # --- per-core slot fill -------------------------------------------------
    nu_cnt = [0] * NCORES
    ni_cnt = [0] * NCORES
    for k in range(NCORES):
        nu_cnt[k] = len({uid[e] for e in core_events[k]})
        ni_cnt[k] = len({iid[e] for e in core_events[k]})
    nu0 = max(nu_cnt)
    ni0 = max(ni_cnt)
    base = nu0 + ni0
    nvcols = base + 2 * ne
    assert nvcols < 32000, nvcols

    vbase = [base + 2 * s_off for (_, _, s_off, _) in steps]

    u_src = np.zeros((NCORES, ne), np.int16)
    i_src = np.zeros((NCORES, ne), np.int16)
    gid = np.full((NCORES, ne), -1, np.int32)
    u_init = [[] for _ in range(NCORES)]   # user ids, first-touch order
    i_init = [[] for _ in range(NCORES)]
    # per (core, step): leading slots whose u / v source is chained
    u_chain_n = np.zeros((NCORES, len(steps)), np.int32)
    v_chain_n = np.zeros((NCORES, len(steps)), np.int32)

    for k in range(NCORES):
        col_u, col_i = {}, {}
        last_su, last_si = {}, {}
        qpos = [0] * nlev
        for s, (l, b, s_off, _) in enumerate(steps):
            q = queues[k][l]
            take = min(b, len(q) - qpos[l])
            for j in range(take):
                e = q[qpos[l] + j]
                slot = s_off + j
                u, i = uid[e], iid[e]
                if u in last_su:
                    u_src[k, slot] = last_su[u]
                    u_chain_n[k, s] = j + 1
                else:
                    c = col_u.setdefault(u, len(col_u))
                    if c == len(u_init[k]):
                        u_init[k].append(u)
                    u_src[k, slot] = c
                if i in last_si:
                    i_src[k, slot] = last_si[i]
                    v_chain_n[k, s] = j + 1
                else:
                    c = col_i.setdefault(i, len(col_i))
                    if c == len(i_init[k]):
                        i_init[k].append(i)
                    i_src[k, slot] = nu0 + c
                last_su[u] = vbase[s] + j
                last_si[i] = vbase[s] + b + j
                gid[k, slot] = e
            qpos[l] += take
        for s, (l, b, s_off, _) in enumerate(steps):
            assert u_src[k, s_off:s_off + b].max(initial=0) < vbase[s]
            assert i_src[k, s_off:s_off + b].max(initial=0) < vbase[s]

    # padded per-step device gather sizes (shared across cores)
    ug_n = [0] * len(steps)
    vg_n = [0] * len(steps)
    for s, (l, b, s_off, _) in enumerate(steps):
        mu = int(u_chain_n[:, s].max())
        mv = int(v_chain_n[:, s].max())
        ug_n[s] = 0 if mu == 0 else min(b, _round16(mu))
        vg_n[s] = 0 if mv == 0 else min(b, _round16(mv))

    sc = _Schedule()
    sc.nev, sc.ne, sc.nu0, sc.ni0 = nev, ne, nu0, ni0
    sc.base, sc.nvcols, sc.nicol = base, nvcols, nicol
    sc.steps = [(l, b, s_off, vbase[s], ic, ug_n[s], vg_n[s])
                for s, (l, b, s_off, ic) in enumerate(steps)]
    sc.u_src, sc.i_src, sc.gid = u_src, i_src, gid
    sc.u_init, sc.i_init = u_init, i_init
    # post-loop chunks aligned to step boundaries, each <= 512 wide
    chunks = []
    cs = 0
    for (l, b, s_off, ic) in steps:
        if s_off + b - cs > 512:
            chunks.append((cs, s_off - cs))
            cs = s_off
    chunks.append((cs, ne - cs))
    sc.chunks = chunks
    return sc


def _wrap_idx(sc, idx):
    """Per-step wrapped idx layout [128, nicol]; step s block at even col."""
    out = np.zeros((16, sc.nicol), np.int16)
    for (_, b, off, _, ic, _, _) in sc.steps:
        w = idx[off:off + b].reshape(b // LANE, LANE).T.astype(np.int16)
        out[:, ic:ic + b // LANE] = w
    return np.tile(out, (8, 1))


def _prep_shared(inp):
    """Weight stacks shared by all cores (fp32r pre-rounded)."""
    f = np.float32
    uwi, uwh = inp["ugru_wi"].astype(f), inp["ugru_wh"].astype(f)
    iwi, iwh = inp["igru_wi"].astype(f), inp["igru_wh"].astype(f)
    t1w, t2w, t3w = inp["t1_w"].astype(f), inp["t2_w"].astype(f), inp["t3_w"].astype(f)

    blocks = []
    for g in (0, 1):                                  # r, z
        s = slice(g * E, (g + 1) * E)
        blocks += [uwi[s].T, uwh[s].T, iwi[s].T, iwh[s].T]
    s = slice(2 * E, 3 * E)
    blocks += [uwi[s].T, iwi[s].T]                    # inn (applied to x)
    blocks += [uwh[s].T, iwh[s].T]                    # hn  (applied to h)
    blocks += [t1w[:, :E].T, t1w[:, E:].T, t2w.T]     # 128,128,32 cols
    wstack = np.concatenate(blocks, axis=1)
    extra = np.zeros((E, 2), f)
    extra[:32, 0] = t3w[0]
    extra[:, 1] = 1.0
    wstack = np.concatenate([wstack, extra], axis=1)  # t3 col, ones col

    ub_i, ub_h = inp["ugru_bi"].astype(f), inp["ugru_bh"].astype(f)
    ib_i, ib_h = inp["igru_bi"].astype(f), inp["igru_bh"].astype(f)
    bstack = np.zeros((2, 4 * E), f)
    bstack[0, 0:E] = ub_i[0:E] + ub_h[0:E]
    bstack[1, 0:E] = ib_i[0:E] + ib_h[0:E]
    bstack[0, E:2 * E] = ub_i[E:2 * E] + ub_h[E:2 * E]
    bstack[1, E:2 * E] = ib_i[E:2 * E] + ib_h[E:2 * E]
    bstack[0, 2 * E:3 * E] = ub_i[2 * E:]
    bstack[1, 2 * E:3 * E] = ib_i[2 * E:]
    bstack[0, 3 * E:] = ub_h[2 * E:]
    bstack[1, 3 * E:] = ib_h[2 * E:]

    bmisc = np.zeros((E, 6), f)
    bmisc[:, 0] = inp["t1_b"].astype(f)
    bmisc[:32, 1] = inp["t2_b"].astype(f)
    bmisc[0, 2] = inp["t3_b"].astype(f)[0]
    bmisc[:, 3] = 1.0
    bmisc[:, 4] = 1e-10
    return _round_fp32r(wstack), _round_fp32r(bstack), bmisc


def _sel_array(sc):
    sel = np.zeros((2, 2 * sc.ne), np.float32)  # 0/1: exact in fp32r
    for (_, b, off, _, _, _, _) in sc.steps:
        sel[0, 2 * off: 2 * off + b] = 1.0
        sel[1, 2 * off + b: 2 * off + 2 * b] = 1.0
    return sel


def _core_inputs(inp, sc, k):
    """Per-core VBUF init, host-prefilled HS staging, gather index arrays."""
    f = np.float32
    vb = np.zeros((E, sc.base), f)
    uu = sc.u_init[k]
    ii = sc.i_init[k]
    if uu:
        vb[:, :len(uu)] = inp["user_emb"][np.asarray(uu)].T.astype(f)
    if ii:
        vb[:, sc.nu0:sc.nu0 + len(ii)] = inp["item_emb"][np.asarray(ii)].T.astype(f)
    vb = _round_fp32r(vb)
    # hs prefill: exactly what a device gather of init-sourced cols returns
    usrc = sc.u_src[k].astype(np.int64)
    isrc = sc.i_src[k].astype(np.int64)
    hsu = np.where(usrc < sc.base, vb[:, np.minimum(usrc, sc.base - 1)], 0.0)
    hsv = np.where(isrc < sc.base, vb[:, np.minimum(isrc, sc.base - 1)], 0.0)
    hs = np.concatenate([hsu, hsv], axis=1).astype(f)
    gu = _wrap_idx(sc, sc.u_src[k])
    gv = _wrap_idx(sc, sc.i_src[k])
    return vb, hs, gu, gv


# ----------------------------------------------------------------------------
# pure-numpy model of the scheduled computation (validation / debugging)
# ----------------------------------------------------------------------------

def _numpy_model(inp, sc):
    wstack, bstack, bmisc = _prep_shared(inp)
    sel = _sel_array(sc)
    ne = sc.ne
    out = np.zeros((sc.nev, 2), np.float32)

    def blk(i):
        return wstack[:, i * E:(i + 1) * E]

    for k in range(NCORES):
        vbinit = _core_inputs(inp, sc, k)[0]
        vb = np.zeros((E, sc.nvcols), np.float32)
        vb[:, :sc.base] = vbinit
        hsu = np.zeros((E, ne), np.float32)
        hsv = np.zeros((E, ne), np.float32)
        for (l, b, off, vbase, _, _, _) in sc.steps:
            ug = vb[:, sc.u_src[k, off:off + b]]
            vg = vb[:, sc.i_src[k, off:off + b]]
            selb = sel[:, 2 * off:2 * off + 2 * b]
            pr = bstack[:, 0:E].T @ selb
            pr[:, :b] += blk(0).T @ vg + blk(1).T @ ug
            pr[:, b:] += blk(2).T @ ug + blk(3).T @ vg
            pz = bstack[:, E:2 * E].T @ selb
            pz[:, :b] += blk(4).T @ vg + blk(5).T @ ug
            pz[:, b:] += blk(6).T @ ug + blk(7).T @ vg
            pinn = bstack[:, 2 * E:3 * E].T @ selb
            pinn[:, :b] += blk(8).T @ vg
            pinn[:, b:] += blk(9).T @ ug
            phn = bstack[:, 3 * E:4 * E].T @ selb
            phn[:, :b] += blk(10).T @ ug
            phn[:, b:] += blk(11).T @ vg
            r = 1.0 / (1.0 + np.exp(-pr))
            z = 1.0 / (1.0 + np.exp(-pz))
            n = np.tanh(pinn + r * phn)
            hcat = np.concatenate([ug, vg], axis=1)
            res = n + z * (hcat - n)
            vb[:, vbase:vbase + 2 * b] = res
            hsu[:, off:off + b] = ug
            hsv[:, off:off + b] = vg
        t1a = wstack[:, 12 * E:13 * E]
        t1b = wstack[:, 13 * E:14 * E]
        t2 = wstack[:, 14 * E:14 * E + 32]
        t3 = wstack[:32, 14 * E + 32]
        h1 = np.maximum(t1a.T @ hsu + t1b.T @ hsv + bmisc[:, 0:1], 0.0)
        h2 = np.maximum(t2.T @ h1 + bmisc[:32, 1:2], 0.0)
        score = 1.0 / (1.0 + np.exp(-(t3 @ h2 + bmisc[0, 2])))
        dot = (hsu * hsv).sum(axis=0)
        l0 = np.log(np.log1p(np.exp(dot)) + 1e-10)
        mask = sc.gid[k] >= 0
        g = sc.gid[k][mask]
        out[g, 0] = -l0[mask]
        out[g, 1] = score[mask]
    return out


# ----------------------------------------------------------------------------
# device program
# ----------------------------------------------------------------------------

def _build_program(sc):
    import concourse.bass as bass
    import concourse.tile as tile
    from concourse import bacc, mybir
    from concourse.tile_rust import add_dep_helper

    f32 = mybir.dt.float32
    f32r = mybir.dt.float32r
    i16 = mybir.dt.int16
    ne = sc.ne
    W = 14 * E + 32 + 2    # wstack cols
    W3 = 14 * E + 32       # t3 col
    WON = W3 + 1           # ones col
    AF = mybir.ActivationFunctionType
    OP = mybir.AluOpType

    nc = bacc.Bacc("TRN2", target_bir_lowering=False, debug=False)
    d_vb = nc.dram_tensor("vbinit", [E, sc.base], f32, kind="ExternalInput").ap()
    d_hs = nc.dram_tensor("hsinit", [E, 2 * ne], f32, kind="ExternalInput").ap()
    d_w = nc.dram_tensor("wstack", [E, W], f32r, kind="ExternalInput").ap()
    d_b = nc.dram_tensor("bstack", [2, 4 * E], f32r, kind="ExternalInput").ap()
    d_sel = nc.dram_tensor("sel", [2, 2 * ne], f32r, kind="ExternalInput").ap()
    d_bm = nc.dram_tensor("bmisc", [E, 6], f32, kind="ExternalInput").ap()
    d_gu = nc.dram_tensor("gu", [E, sc.nicol], i16, kind="ExternalInput").ap()
    d_gv = nc.dram_tensor("gv", [E, sc.nicol], i16, kind="ExternalInput").ap()
    d_outl = nc.dram_tensor("outl", [1, ne], f32, kind="ExternalOutput").ap()
    d_outs = nc.dram_tensor("outs", [1, ne], f32, kind="ExternalOutput").ap()

    with tile.TileContext(nc) as tc, ExitStack() as ctx:
        const = ctx.enter_context(tc.tile_pool(name="const", bufs=1))
        psum = ctx.enter_context(tc.tile_pool(name="psum", bufs=2, space="PSUM"))
        work = ctx.enter_context(tc.tile_pool(name="work", bufs=2))

        # dummy gather issued first: pulls the ext-isa GPSIMD library into
        # IRAM (~9us) while the input DMAs stream in parallel.
        warm = const.tile([E, 16], f32)
        nc.vector.memset(warm[:], 0.0)
        warmi = const.tile([E, 2], i16)
        nc.vector.memset(warmi[:].bitcast(f32), 0.0)
        warmo = const.tile([E, 16], f32)
        nc.gpsimd.ap_gather(warmo[:], warm[:], warmi[:, 0:1],
                            channels=E, num_elems=16, d=1, num_idxs=16)

        vbuf = const.tile([E, sc.nvcols], f32)
        nc.sync.dma_start(vbuf[:, :sc.base], d_vb[:])
        nc.vector.memset(vbuf[:, sc.base:], 0.0)
        hs = const.tile([E, 2 * ne], f32)
        nc.sync.dma_start(hs[:], d_hs[:])
        hs_r = const.tile([E, 2 * ne], f32r)
        # host hs data is pre-rounded: plain on-device copy doubles as the
        # initial fp32r mirror (DVE CAST, rounds again -- idempotent)
        nc.vector.tensor_copy(out=hs_r[:], in_=hs[:])
        wsb = const.tile([E, W], f32r)
        nc.sync.dma_start(wsb[:], d_w[:])
        bsb = const.tile([2, 4 * E], f32r)
        nc.sync.dma_start(bsb[:], d_b[:])
        selsb = const.tile([2, 2 * ne], f32r)
        nc.sync.dma_start(selsb[:], d_sel[:])
        bmsb = const.tile([E, 6], f32)
        nc.sync.dma_start(bmsb[:], d_bm[:])
        gu = const.tile([E, sc.nicol], i16)
        nc.sync.dma_start(gu[:], d_gu[:])
        gv = const.tile([E, sc.nicol], i16)
        nc.sync.dma_start(gv[:], d_gv[:])
        losssb = const.tile([1, ne], f32)
        scoresb = const.tile([1, ne], f32)

        def mm(out_ap, wcol, rhs_ap, start, stop):
            nc.tensor.matmul(
                out_ap,
                lhsT=wsb[:, wcol * E:(wcol + 1) * E],
                rhs=rhs_ap,
                start=start, stop=stop, skip_group_check=True,
            )

        wb_prev = None
        for (l, b, off, vbase, ic, un, vn) in sc.steps:
            # device gathers only for the chained prefix of the step
            for (cnt, dst, idxt) in ((un, off, gu), (vn, ne + off, gv)):
                if cnt == 0:
                    continue
                g = nc.gpsimd.ap_gather(
                    hs[:, dst:dst + cnt], vbuf[:], idxt[:, ic:ic + cnt // LANE],
                    channels=E, num_elems=sc.nvcols, d=1, num_idxs=cnt)
                if wb_prev is not None:
                    add_dep_helper(g.ins, wb_prev.ins,
                                   reason="gather reads prev writeback")
                nc.vector.tensor_copy(out=hs_r[:, dst:dst + cnt],
                                      in_=hs[:, dst:dst + cnt])
            ug = hs_r[:, off:off + b]
            vg = hs_r[:, ne + off:ne + off + b]
            selb = selsb[:, 2 * off:2 * off + 2 * b]

            pr = psum.tile([E, 2 * b], f32, tag="pr")
            pz = psum.tile([E, 2 * b], f32, tag="pz")
            pinn = psum.tile([E, 2 * b], f32, tag="pinn")
            phn = psum.tile([E, 2 * b], f32, tag="phn")

            # user cell: x = v, h = u ; item cell: x = u, h = v
            plan = (
                (pr, 0, ((0, vg), (1, ug)), ((2, ug), (3, vg))),
                (pz, 1, ((4, vg), (5, ug)), ((6, ug), (7, vg))),
                (pinn, 2, ((8, vg),), ((9, ug),)),
                (phn, 3, ((10, ug),), ((11, vg),)),
            )
            for (pt, bcol, left, right) in plan:
                nc.tensor.matmul(
                    pt[:, 0:2 * b],
                    lhsT=bsb[:, bcol * E:(bcol + 1) * E],
                    rhs=selb, start=True, stop=False, skip_group_check=True)
                for wc, rh in left:
                    mm(pt[:, 0:b], wc, rh, False, False)
                for n_, (wc, rh) in enumerate(right):
                    mm(pt[:, b:2 * b], wc, rh, False, n_ == len(right) - 1)

            r = work.tile([E, 2 * b], f32, tag="r")
            z = work.tile([E, 2 * b], f32, tag="z")
            nfn = work.tile([E, 2 * b], f32, tag="nfn")
            tmp = work.tile([E, 2 * b], f32, tag="tmp")
            nc.scalar.activation(r[:], pr[:], AF.Sigmoid, bias=bmsb[:, 5:6])
            nc.scalar.activation(z[:], pz[:], AF.Sigmoid, bias=bmsb[:, 5:6])
            nc.vector.tensor_tensor(out=tmp[:], in0=r[:], in1=phn[:], op=OP.mult)
            nc.vector.tensor_tensor(out=tmp[:], in0=tmp[:], in1=pinn[:], op=OP.add)
            nc.scalar.activation(nfn[:], tmp[:], AF.Tanh, bias=bmsb[:, 5:6])
            # d = hcat - n ; hcat = [ug | vg] = strided [128, 2, b] view of hs
            hcat3 = hs[:].rearrange("p (t x) -> p t x", t=2)[:, :, off:off + b]
            d3 = tmp[:].rearrange("p (t x) -> p t x", t=2)
            n3 = nfn[:].rearrange("p (t x) -> p t x", t=2)
            nc.vector.tensor_tensor(out=d3, in0=hcat3, in1=n3, op=OP.subtract)
            nc.vector.tensor_tensor(out=tmp[:], in0=z[:], in1=tmp[:], op=OP.mult)
            wb_prev = nc.vector.tensor_tensor(
                out=vbuf[:, vbase:vbase + 2 * b],
                in0=nfn[:], in1=tmp[:], op=OP.add)

        # ---- post loop: MLP + loss for all events (step-aligned chunks) ----
        for (c0, cb) in sc.chunks:
            u_c = hs_r[:, c0:c0 + cb]
            v_c = hs_r[:, ne + c0:ne + c0 + cb]
            h1p = psum.tile([E, cb], f32, tag="pr")
            mm(h1p[:], 12, u_c, True, False)
            mm(h1p[:], 13, v_c, False, True)
            h1 = work.tile([E, cb], f32r, tag="r")
            nc.scalar.activation(h1[:], h1p[:], AF.Relu, bias=bmsb[:, 0:1])
            h2p = psum.tile([32, cb], f32, tag="pz")
            nc.tensor.matmul(h2p[:], lhsT=wsb[:, 14 * E:14 * E + 32],
                             rhs=h1[:], start=True, stop=True,
                             skip_group_check=True)
            h2 = work.tile([32, cb], f32r, tag="z")
            nc.scalar.activation(h2[:], h2p[:], AF.Relu, bias=bmsb[:32, 1:2])
            h3p = psum.tile([1, cb], f32, tag="pinn")
            nc.tensor.matmul(h3p[:], lhsT=wsb[:32, W3:W3 + 1],
                             rhs=h2[:], start=True, stop=True,
                             skip_group_check=True)
            nc.scalar.activation(scoresb[:, c0:c0 + cb], h3p[:], AF.Sigmoid,
                                 bias=bmsb[0:1, 2:3])
            uvm = work.tile([E, cb], f32r, tag="nfn")
            nc.vector.tensor_tensor(out=uvm[:], in0=hs[:, c0:c0 + cb],
                                    in1=hs[:, ne + c0:ne + c0 + cb], op=OP.mult)
            dotp = psum.tile([1, cb], f32, tag="phn")
            nc.tensor.matmul(dotp[:], lhsT=wsb[:, WON:WON + 1],
                             rhs=uvm[:], start=True, stop=True,
                             skip_group_check=True)
            ex = work.tile([1, cb], f32, tag="ex")
            nc.scalar.activation(ex[:], dotp[:], AF.Exp, bias=bmsb[0:1, 5:6])
            sp = work.tile([1, cb], f32, tag="sp")
            nc.scalar.activation(sp[:], ex[:], AF.Ln, bias=bmsb[0:1, 3:4])
            nc.scalar.activation(losssb[:, c0:c0 + cb], sp[:], AF.Ln,
                                 bias=bmsb[0:1, 4:5])

        nc.sync.dma_start(d_outl[:], losssb[:])
        nc.sync.dma_start(d_outs[:], scoresb[:])

    nc.compile()
    return nc


# ----------------------------------------------------------------------------
# entry point
# ----------------------------------------------------------------------------

def kernel(**inputs):
    global LAST_EXEC_NS
    from concourse.bass_utils import run_bass_kernel_spmd

    uid = np.asarray(inputs["user_ids"])
    iid = np.asarray(inputs["item_ids"])
    key = (uid.tobytes(), iid.tobytes())
    if key not in _CACHE:
        sc = _build_schedule(uid, iid)
        nc = _build_program(sc)
        _CACHE[key] = (sc, nc)
    sc, nc = _CACHE[key]

    wstack, bstack, bmisc = _prep_shared(inputs)
    sel = _sel_array(sc)
    in_maps = []
    for k in range(NCORES):
        vb, hsi, gu, gv = _core_inputs(inputs, sc, k)
        in_maps.append({
            "vbinit": vb, "hsinit": hsi,
            "wstack": wstack, "bstack": bstack, "sel": sel,
            "bmisc": bmisc, "gu": gu, "gv": gv,
        })

    res = run_bass_kernel_spmd(nc, in_maps, list(range(NCORES)), trace=TRACE)
    LAST_EXEC_NS = res.exec_time_ns

    out = np.zeros((sc.nev, 2), np.float32)
    for k in range(NCORES):
        mask = sc.gid[k] >= 0
        g = sc.gid[k][mask]
        out[g, 0] = -res.results[k]["outl"][0, mask]
        out[g, 1] = res.results[k]["outs"][0, mask]
    return out



# revision 3
# speedup vs baseline: 1.5945x; 1.5945x over previous
"""DeepCoevolve on Trainium2 (Bass/Tile), 8 NeuronCores — v2.

Key observations driving this design
------------------------------------
1. The reference returns only per-event (loss, score); the final embedding
   tables are discarded.  So the GRU updates matter only for events whose
   user/item row is read again later ("parents": ~232 of 4096).  Everything
   else is a pure feed-forward MLP + dot on the *initial* (host-gathered)
   embeddings.
2. All activations used (sigmoid, tanh, relu, copy) live in ONE ACT table
   set; the loss's exp/ln do not.  The loss -ln(softplus(d)+1e-10) is
   evaluated as a degree-6 polynomial in d (|d| < 0.12 by construction,
   fit on [-0.35, 0.35], max err 4e-10), so the single table set is loaded
   once, warmed during the input DMA.
3. Scores and dots are computed TRANSPOSED (events on partitions) via
   lhsT=data matmuls, so the final sigmoid/polynomial run 128-wide.
4. bf16 matmul operands everywhere (FWL weight loads, 2x matmul rate,
   halved DMA); f32 PSUM accumulation and f32 GRU element-wise state.
   rel-err budget is 2e-2; bf16 lands ~1e-3.
5. Startup: inputs packed into 2 DMA triggers (each trigger costs ~650ns
   serial on the sync sequencer); ACT table + GPSIMD gather library warmed
   up front.

Structure per core (slots = events, padded; schedule shared by all cores):
  level 0: slots [0, 512)  (parents first), all inputs host-prefilled
  level l: slots [off_l, off_l+L_l), inputs ap_gather'ed from VT
           (parent GRU outputs for chained entities, own-slot prefill
           otherwise), then cast to the bf16 mirror HSB
  GRU step per level with parents (width BP_l): 12 bf16 matmuls -> 4 PSUM
  gate tiles, ACT-bias sigmoids/tanh (biases folded via per-partition
  bias columns, not matmuls), f32 update written to VT for later gathers
  MLP/dot chunks of 128 slots interleaved between chain steps to keep the
  PE busy during gather latency.
"""

import numpy as np
import ml_dtypes
from contextlib import ExitStack

E = 128
NCORES = 8
L0 = 512                     # level-0 slot region (multiple of 128)
BF = ml_dtypes.bfloat16

_CACHE = {}
LAST_EXEC_NS = None
TRACE = False

# P(d) ~= -ln(ln(1+e^d)+1e-10), Chebyshev fit on [-0.35, 0.35]
_PC = [0.3665129204487056, -0.7213475179713207, 0.07983423913776333,
       0.004969524189519166, -0.002373002518832123,
       -0.00024914267712346905, 0.00013386182178660658]

# blob byte offsets (per partition)
OFF_W = 0                    # 1824 cols bf16
OFF_MISC = 3648              # 2 cols bf16 (t3 col, ones col)
OFF_B = 3652                 # 12 cols f32 (bias columns)
OFF_IDX = 3700               # NIC cols int16


def _r16(x):
    return max(16, (int(x) + 15) // 16 * 16)


class _S:
    pass


# ----------------------------------------------------------------------------
# host-side scheduling
# ----------------------------------------------------------------------------

def _build_schedule(uid, iid):
    uid = np.asarray(uid, np.int64)
    iid = np.asarray(iid, np.int64)
    nev = len(uid)

    lvl = np.zeros(nev, np.int32)
    ispar = np.zeros(nev, bool)
    last_u, last_i = {}, {}
    par = list(range(nev))

    def find(x):
        while par[x] != x:
            par[x] = par[par[x]]
            x = par[x]
        return x

    for e in range(nev):
        l = 0
        a = last_u.get(uid[e])
        b = last_i.get(iid[e])
        if a is not None:
            l = lvl[a] + 1
            ispar[a] = True
            ra, rb = find(e), find(a)
            if ra != rb:
                par[ra] = rb
        if b is not None:
            l = max(l, lvl[b] + 1)
            ispar[b] = True
            ra, rb = find(e), find(b)
            if ra != rb:
                par[ra] = rb
        lvl[e] = l
        last_u[uid[e]] = e
        last_i[iid[e]] = e
    nlev = int(lvl.max()) + 1

    comps = {}
    for e in range(nev):
        comps.setdefault(find(e), []).append(e)
    comp_list = sorted(comps.values(), key=len, reverse=True)
    core_events = [[] for _ in range(NCORES)]
    tot = [0] * NCORES
    for c in comp_list:
        k = min(range(NCORES), key=lambda i: tot[i])
        core_events[k].extend(c)
        tot[k] += len(c)

    queues = [[[] for _ in range(nlev)] for _ in range(NCORES)]
    for k in range(NCORES):
        for e in sorted(core_events[k]):
            queues[k][lvl[e]].append(e)
        for l in range(nlev):
            queues[k][l].sort(key=lambda e: (not ispar[e], e))

    assert max(len(queues[k][0]) for k in range(NCORES)) <= L0
    L = [L0] + [_r16(max(len(queues[k][l]) for k in range(NCORES)))
                for l in range(1, nlev)]
    BP = []
    for l in range(nlev):
        bp = max(sum(1 for e in queues[k][l] if ispar[e])
                 for k in range(NCORES))
        BP.append((bp + 3) // 4 * 4 if bp else 0)
    off = [0] * nlev
    for l in range(1, nlev):
        off[l] = off[l - 1] + L[l - 1]
    ne = off[-1] + L[-1]
    NCH = (ne + 127) // 128
    SB = {}
    cum = 2 * ne
    for l in range(nlev):
        if BP[l]:
            SB[l] = cum
            cum += 2 * BP[l]
    NV = cum
    assert NV < 32000

    gid = np.full((NCORES, ne), -1, np.int32)
    pre_u = np.full((NCORES, ne), -1, np.int64)
    pre_v = np.full((NCORES, ne), -1, np.int64)
    usrc = np.zeros((NCORES, ne), np.int32)
    vsrc = np.zeros((NCORES, ne), np.int32)
    for k in range(NCORES):
        lsu, lsi = {}, {}
        for l in range(nlev):
            q = queues[k][l]
            assert len(q) <= L[l]
            for j in range(L[l]):
                s = off[l] + j
                if j < len(q):
                    e = q[j]
                    gid[k, s] = e
                    u, i = uid[e], iid[e]
                    if u in lsu:
                        usrc[k, s] = lsu[u]
                    else:
                        usrc[k, s] = s
                        pre_u[k, s] = u
                    if i in lsi:
                        vsrc[k, s] = lsi[i]
                    else:
                        vsrc[k, s] = ne + s
                        pre_v[k, s] = i
                    if ispar[e]:
                        assert j < BP[l]
                        lsu[u] = SB[l] + j
                        lsi[i] = SB[l] + BP[l] + j
                else:
                    usrc[k, s] = s
                    vsrc[k, s] = ne + s

    # gather index columns (levels >= 1), int16 wrapped into 16 partitions
    ic = [0] * nlev
    cols = 0
    for l in range(1, nlev):
        ic[l] = cols
        cols += (2 * L[l]) // 16
    NIC = max(cols, 2)
    assert NIC % 2 == 0
    gidx = np.zeros((NCORES, 16, NIC), np.int16)
    for k in range(NCORES):
        for l in range(1, nlev):
            lanes = np.concatenate(
                [usrc[k, off[l]:off[l] + L[l]],
                 vsrc[k, off[l]:off[l] + L[l]]]).astype(np.int16)
            gidx[k, :, ic[l]:ic[l] + len(lanes) // 16] = \
                lanes.reshape(-1, 16).T

    sc = _S()
    sc.nev, sc.ne, sc.nlev, sc.NCH, sc.NV, sc.NIC = nev, ne, nlev, NCH, NV, NIC
    sc.L, sc.BP, sc.off, sc.SB, sc.ic = L, BP, off, SB, ic
    sc.gid, sc.pre_u, sc.pre_v = gid, pre_u, pre_v
    sc.usrc, sc.vsrc, sc.gidx = usrc, vsrc, gidx
    sc.chunks = [(c * 128, min(128, ne - c * 128), c) for c in range(NCH)]
    sc.BA = OFF_IDX + 2 * NIC + 4 * ne
    return sc


# ----------------------------------------------------------------------------
# host-side data prep
# ----------------------------------------------------------------------------

def _prep_weights(inp):
    f = np.float32
    uwi, uwh = inp["ugru_wi"].astype(f), inp["ugru_wh"].astype(f)
    iwi, iwh = inp["igru_wi"].astype(f), inp["igru_wh"].astype(f)
    t1w, t2w, t3w = (inp["t1_w"].astype(f), inp["t2_w"].astype(f),
                     inp["t3_w"].astype(f))
    blocks = []
    for g in (0, 1):                                  # r, z gates
        s = slice(g * E, (g + 1) * E)
        blocks += [uwi[s].T, uwh[s].T, iwi[s].T, iwh[s].T]
    s = slice(2 * E, 3 * E)
    blocks += [uwi[s].T, iwi[s].T]                    # inn (applied to x)
    blocks += [uwh[s].T, iwh[s].T]                    # hn  (applied to h)
    blocks += [t1w[:, :E].T, t1w[:, E:].T, t2w.T]
    wstack = np.concatenate(blocks, axis=1)           # [E, 1824]

    misc = np.zeros((E, 2), f)
    misc[:32, 0] = t3w[0]
    misc[:, 1] = 1.0

    ubi, ubh = inp["ugru_bi"].astype(f), inp["ugru_bh"].astype(f)
    ibi, ibh = inp["igru_bi"].astype(f), inp["igru_bh"].astype(f)
    B = np.zeros((E, 12), f)
    B[:, 0] = ubi[0:E] + ubh[0:E]          # r user
    B[:, 1] = ibi[0:E] + ibh[0:E]          # r item
    B[:, 2] = ubi[E:2 * E] + ubh[E:2 * E]  # z user
    B[:, 3] = ibi[E:2 * E] + ibh[E:2 * E]  # z item
    B[:, 4] = ubi[2 * E:]                  # inn user (tanh bias)
    B[:, 5] = ibi[2 * E:]                  # inn item
    B[:, 6] = ubh[2 * E:]                  # hn user (copy bias)
    B[:, 7] = ibh[2 * E:]                  # hn item
    B[:, 8] = inp["t1_b"].astype(f)
    B[:32, 9] = inp["t2_b"].astype(f)
    B[:, 10] = inp["t3_b"].astype(f)[0]    # replicated t3 bias
    return wstack, misc, B


def _core_hs(inp, sc, k):
    """f32 prefill [E, 2ne]: first-touch embeddings, zeros elsewhere."""
    ne = sc.ne
    hs = np.zeros((E, 2 * ne), np.float32)
    mu = sc.pre_u[k] >= 0
    if mu.any():
        hs[:, 0:ne][:, mu] = inp["user_emb"][sc.pre_u[k][mu]].T
    mv = sc.pre_v[k] >= 0
    if mv.any():
        hs[:, ne:][:, mv] = inp["item_emb"][sc.pre_v[k][mv]].T
    return hs


def _core_blob(sc, k, wbf, miscbf, B, hs):
    ne = sc.ne
    blob = np.zeros((E, sc.BA), np.uint8)
    blob[:, OFF_W:OFF_W + 3648] = np.ascontiguousarray(wbf).view(np.uint8)
    blob[:, OFF_MISC:OFF_MISC + 4] = np.ascontiguousarray(miscbf).view(np.uint8)
    blob[:, OFF_B:OFF_B + 48] = np.ascontiguousarray(B).view(np.uint8)
    idx = np.tile(sc.gidx[k], (8, 1))                 # [128, NIC]
    blob[:, OFF_IDX:OFF_IDX + 2 * sc.NIC] = \
        np.ascontiguousarray(idx).view(np.uint8)
    hsb = np.ascontiguousarray(hs.astype(BF))
    blob[:, OFF_IDX + 2 * sc.NIC:] = hsb.view(np.uint8)
    return blob


# ----------------------------------------------------------------------------
# numpy model of the scheduled computation (host validation)
# ----------------------------------------------------------------------------

def _numpy_model(inp, sc):
    wstack, misc, B = _prep_weights(inp)
    ne, nlev = sc.ne, sc.nlev
    out = np.zeros((sc.nev, 2), np.float32)

    def blk(i):
        return wstack[:, i * E:(i + 1) * E]

    def sig(x):
        return 1.0 / (1.0 + np.exp(-x))

    for k in range(NCORES):
        hs = _core_hs(inp, sc, k)
        vt = np.zeros((E, sc.NV), np.float32)
        vt[:, 0:2 * ne] = hs
        for l in range(nlev):
            o, Ll = sc.off[l], sc.L[l]
            if l > 0:
                hs[:, o:o + Ll] = vt[:, sc.usrc[k, o:o + Ll]]
                hs[:, ne + o:ne + o + Ll] = vt[:, sc.vsrc[k, o:o + Ll]]
            b = sc.BP[l]
            if not b:
                continue
            ug = hs[:, o:o + b]
            vg = hs[:, ne + o:ne + o + b]
            r = sig(blk(0).T @ vg + blk(1).T @ ug + B[:, 0:1])
            z = sig(blk(4).T @ vg + blk(5).T @ ug + B[:, 2:3])
            n = np.tanh(blk(8).T @ vg + B[:, 4:5]
                        + r * (blk(10).T @ ug + B[:, 6:7]))
            hu = n + z * (ug - n)
            r2 = sig(blk(2).T @ ug + blk(3).T @ vg + B[:, 1:2])
            z2 = sig(blk(6).T @ ug + blk(7).T @ vg + B[:, 3:4])
            n2 = np.tanh(blk(9).T @ ug + B[:, 5:6]
                         + r2 * (blk(11).T @ vg + B[:, 7:8]))
            hv = n2 + z2 * (vg - n2)
            sb = sc.SB[l]
            vt[:, sb:sb + b] = hu
            vt[:, sb + b:sb + 2 * b] = hv
        hsu, hsv = hs[:, 0:ne], hs[:, ne:]
        t1a = wstack[:, 12 * E:13 * E]
        t1b = wstack[:, 13 * E:14 * E]
        t2 = wstack[:, 14 * E:14 * E + 32]
        h1 = np.maximum(t1a.T @ hsu + t1b.T @ hsv + B[:, 8:9], 0.0)
        h2 = np.maximum(t2.T @ h1 + B[:32, 9:10], 0.0)
        sco = sig(misc[:32, 0] @ h2 + B[0, 10])
        d = (hsu * hsv).sum(axis=0)
        p = np.full_like(d, _PC[6])
        for c in _PC[5::-1]:
            p = p * d + c
        mask = sc.gid[k] >= 0
        g = sc.gid[k][mask]
        out[g, 0] = p[mask]
        out[g, 1] = sco[mask]
    return out


# ----------------------------------------------------------------------------
# device program
# ----------------------------------------------------------------------------

def _build_program(sc):
    import concourse.bass as bass  # noqa: F401
    import concourse.tile as tile
    from concourse import bacc, mybir
    from concourse.tile_rust import add_dep_helper

    f32 = mybir.dt.float32
    bf16 = mybir.dt.bfloat16
    i16 = mybir.dt.int16
    u8 = mybir.dt.uint8
    AF = mybir.ActivationFunctionType
    OP = mybir.AluOpType
    ne, NV, NCH, nlev = sc.ne, sc.NV, sc.NCH, sc.nlev

    nc = bacc.Bacc("TRN2", target_bir_lowering=False, debug=False)
    d_blob = nc.dram_tensor("blob", [E, sc.BA], u8, kind="ExternalInput").ap()
    d_vt = nc.dram_tensor("vthalf", [E, 2 * ne], f32,
                          kind="ExternalInput").ap()
    d_out = nc.dram_tensor("outg", [128, 2 * NCH], f32,
                           kind="ExternalOutput").ap()

    with tile.TileContext(nc) as tc, ExitStack() as ctx:
        const = ctx.enter_context(tc.tile_pool(name="const", bufs=1))
        psA = ctx.enter_context(tc.tile_pool(name="psA", bufs=2, space="PSUM"))
        psB = ctx.enter_context(tc.tile_pool(name="psB", bufs=1, space="PSUM"))
        acc = ctx.enter_context(tc.tile_pool(name="acc", bufs=1, space="PSUM"))
        work = ctx.enter_context(tc.tile_pool(name="work", bufs=2))

        # --- warmups: ACT table set + GPSIMD gather library, during DMA ---
        wtab = const.tile([E, 2], f32)
        nc.vector.memset(wtab[:], 0.0)
        nc.scalar.activation(wtab[:, 1:2], wtab[:, 0:1], AF.Sigmoid, bias=0.0)
        warm = const.tile([E, 16], f32)
        nc.vector.memset(warm[:], 0.0)
        warmi = const.tile([E, 2], i16)
        nc.vector.memset(warmi[:].bitcast(f32), 0.0)
        warmo = const.tile([E, 16], f32)
        nc.gpsimd.ap_gather(warmo[:], warm[:], warmi[:, 0:1],
                            channels=E, num_elems=16, d=1, num_idxs=16)

        # --- input DMA: 2 triggers ---
        blob = const.tile([E, sc.BA], u8)
        vt = const.tile([E, NV], f32)
        dma1 = nc.sync.dma_start(blob[:], d_blob[:])
        dma2 = nc.sync.dma_start(vt[:, 0:2 * ne], d_vt[:])

        wsb = blob[:, OFF_W:OFF_W + 3648].bitcast(bf16)
        miscb = blob[:, OFF_MISC:OFF_MISC + 4].bitcast(bf16)
        bias = blob[:, OFF_B:OFF_B + 48].bitcast(f32)
        idxt = blob[:, OFF_IDX:OFF_IDX + 2 * sc.NIC].bitcast(i16)
        hsb = blob[:, OFF_IDX + 2 * sc.NIC:].bitcast(bf16)

        SW = max(max(sc.L[1:], default=16), sc.BP[0])
        stag = const.tile([E, 2 * SW], f32)
        outsb = const.tile([128, 2 * NCH], f32)
        pscore = acc.tile([128, NCH], f32, tag="pscore")
        pdot = acc.tile([128, NCH], f32, tag="pdot")

        def mmw(out_ap, col, ncols, rhs, start, stop):
            nc.tensor.matmul(out_ap, lhsT=wsb[:, col:col + ncols], rhs=rhs,
                             start=start, stop=stop, skip_group_check=True)

        def gru_step(l, sw):
            """GRU for the BP[l] parent slots of level l.  Inputs: bf16
            mirrors in hsb, f32 originals in stag (width sw per side)."""
            b = sc.BP[l]
            o = sc.off[l]
            ug = hsb[:, o:o + b]
            vg = hsb[:, ne + o:ne + o + b]
            pr = psA.tile([E, 2 * b], f32, tag="pr")
            pz = psA.tile([E, 2 * b], f32, tag="pz")
            pinn = psB.tile([E, 2 * b], f32, tag="pinn")
            phn = psB.tile([E, 2 * b], f32, tag="phn")
            # user cell: x=v, h=u ; item cell: x=u, h=v
            mmw(pr[:, 0:b], 0 * E, E, vg, True, False)
            mmw(pr[:, 0:b], 1 * E, E, ug, False, True)
            mmw(pr[:, b:2 * b], 2 * E, E, ug, True, False)
            mmw(pr[:, b:2 * b], 3 * E, E, vg, False, True)
            mmw(phn[:, 0:b], 10 * E, E, ug, True, True)
            mmw(phn[:, b:2 * b], 11 * E, E, vg, True, True)
            mmw(pz[:, 0:b], 4 * E, E, vg, True, False)
            mmw(pz[:, 0:b], 5 * E, E, ug, False, True)
            mmw(pz[:, b:2 * b], 6 * E, E, ug, True, False)
            mmw(pz[:, b:2 * b], 7 * E, E, vg, False, True)
            mmw(pinn[:, 0:b], 8 * E, E, vg, True, True)
            mmw(pinn[:, b:2 * b], 9 * E, E, ug, True, True)

            r = work.tile([E, 2 * b], f32, tag="r")
            z = work.tile([E, 2 * b], f32, tag="z")
            ph = work.tile([E, 2 * b], f32, tag="ph")
            nf = work.tile([E, 2 * b], f32, tag="nf")
            tmp = work.tile([E, 2 * b], f32, tag="tmp")
            nc.scalar.activation(r[:, 0:b], pr[:, 0:b], AF.Sigmoid,
                                 bias=bias[:, 0:1])
            nc.scalar.activation(r[:, b:2 * b], pr[:, b:2 * b], AF.Sigmoid,
                                 bias=bias[:, 1:2])
            nc.scalar.activation(ph[:, 0:b], phn[:, 0:b], AF.Identity,
                                 bias=bias[:, 6:7])
            nc.scalar.activation(ph[:, b:2 * b], phn[:, b:2 * b], AF.Identity,
                                 bias=bias[:, 7:8])
            nc.scalar.activation(z[:, 0:b], pz[:, 0:b], AF.Sigmoid,
                                 bias=bias[:, 2:3])
            nc.scalar.activation(z[:, b:2 * b], pz[:, b:2 * b], AF.Sigmoid,
                                 bias=bias[:, 3:4])
            nc.vector.tensor_tensor(out=tmp[:], in0=r[:], in1=ph[:],
                                    op=OP.mult)
            nc.vector.tensor_tensor(out=tmp[:], in0=tmp[:], in1=pinn[:],
                                    op=OP.add)
            nc.scalar.activation(nf[:, 0:b], tmp[:, 0:b], AF.Tanh,
                                 bias=bias[:, 4:5])
            nc.scalar.activation(nf[:, b:2 * b], tmp[:, b:2 * b], AF.Tanh,
                                 bias=bias[:, 5:6])
            # h' = n + z*(h - n); h = [u | v] f32 originals in stag
            hcat3 = stag[:, 0:2 * sw].rearrange(
                "p (t x) -> p t x", t=2)[:, :, 0:b]
            d3 = tmp[:].rearrange("p (t x) -> p t x", t=2)
            n3 = nf[:].rearrange("p (t x) -> p t x", t=2)
            nc.vector.tensor_tensor(out=d3, in0=hcat3, in1=n3,
                                    op=OP.subtract)
            nc.vector.tensor_tensor(out=tmp[:], in0=z[:], in1=tmp[:],
                                    op=OP.mult)
            sb = sc.SB[l]
            return nc.vector.tensor_tensor(out=vt[:, sb:sb + 2 * b],
                                           in0=nf[:], in1=tmp[:], op=OP.add)

        def chunk(c0, cb, cc):
            ub = hsb[:, c0:c0 + cb]
            vb = hsb[:, ne + c0:ne + c0 + cb]
            h1p = psA.tile([E, cb], f32, tag="pr")
            mmw(h1p[:], 12 * E, E, ub, True, False)
            mmw(h1p[:], 13 * E, E, vb, False, True)
            h1 = work.tile([E, cb], bf16, tag="h1")
            nc.scalar.activation(h1[:], h1p[:], AF.Relu, bias=bias[:, 8:9])
            h2p = psA.tile([32, cb], f32, tag="pz")
            mmw(h2p[:], 14 * E, 32, h1[:], True, True)
            h2 = work.tile([32, cb], bf16, tag="h2")
            nc.scalar.activation(h2[:], h2p[:], AF.Relu,
                                 bias=bias[0:32, 9:10])
            nc.tensor.matmul(pscore[0:cb, cc:cc + 1], lhsT=h2[:],
                             rhs=miscb[0:32, 0:1], start=True, stop=True,
                             skip_group_check=True)
            uvm = work.tile([E, cb], bf16, tag="uvm")
            nc.vector.tensor_tensor(out=uvm[:], in0=ub, in1=vb, op=OP.mult)
            nc.tensor.matmul(pdot[0:cb, cc:cc + 1], lhsT=uvm[:],
                             rhs=miscb[:, 1:2], start=True, stop=True,
                             skip_group_check=True)

        # --- step A: level-0 parents (inputs are host-prefilled) ---
        b0 = sc.BP[0]
        anchor = None
        if b0:
            nc.vector.tensor_copy(out=stag[:, 0:b0], in_=hsb[:, 0:b0])
            nc.vector.tensor_copy(out=stag[:, b0:2 * b0],
                                  in_=hsb[:, ne:ne + b0])
            anchor = gru_step(0, b0)

        pure = [c for c in sc.chunks if c[0] + c[1] <= L0]
        rest = [c for c in sc.chunks if c[0] + c[1] > L0]
        pi = 0

        # --- chain levels: gather -> cast -> (GRU) ; interleave chunks ---
        for l in range(1, nlev):
            Ll = sc.L[l]
            o = sc.off[l]
            g = nc.gpsimd.ap_gather(
                stag[:, 0:2 * Ll], vt[:], idxt[:, sc.ic[l]:sc.ic[l] + 2 * Ll // 16],
                channels=E, num_elems=NV, d=1, num_idxs=2 * Ll)
            if anchor is not None:
                add_dep_helper(g.ins, anchor.ins,
                               reason="gather reads prev writeback")
            add_dep_helper(g.ins, dma2.ins, reason="gather reads vt dma")
            add_dep_helper(g.ins, dma1.ins, reason="gather reads idx dma")
            c1 = nc.vector.tensor_copy(out=hsb[:, o:o + Ll],
                                       in_=stag[:, 0:Ll])
            c2 = nc.vector.tensor_copy(out=hsb[:, ne + o:ne + o + Ll],
                                       in_=stag[:, Ll:2 * Ll])
            add_dep_helper(c1.ins, g.ins, reason="cast reads gather out")
            add_dep_helper(c2.ins, g.ins, reason="cast reads gather out")
            if pi < len(pure):
                chunk(*pure[pi])
                pi += 1
            anchor = gru_step(l, Ll) if sc.BP[l] else c2
        while pi < len(pure):
            chunk(*pure[pi])
            pi += 1
        for c in rest:
            chunk(*c)

        # --- scores + polynomial losses, 128-wide ---
        nc.scalar.activation(outsb[:, NCH:2 * NCH], pscore[:], AF.Sigmoid,
                             bias=bias[:, 10:11])
        pt = const.tile([128, NCH], f32)
        nc.vector.tensor_scalar(out=pt[:], in0=pdot[:], scalar1=_PC[6],
                                scalar2=_PC[5], op0=OP.mult, op1=OP.add)
        for k in range(4, -1, -1):
            nc.vector.tensor_tensor(out=pt[:], in0=pt[:], in1=pdot[:],
                                    op=OP.mult)
            dst = outsb[:, 0:NCH] if k == 0 else pt[:]
            nc.vector.tensor_scalar(out=dst, in0=pt[:], scalar1=_PC[k],
                                    scalar2=None, op0=OP.add)
        nc.sync.dma_start(d_out[:], outsb[:])

    nc.compile()
    return nc


# ----------------------------------------------------------------------------
# entry point
# ----------------------------------------------------------------------------

def kernel(**inputs):
    global LAST_EXEC_NS
    from concourse.bass_utils import run_bass_kernel_spmd

    uid = np.asarray(inputs["user_ids"])
    iid = np.asarray(inputs["item_ids"])
    key = (uid.tobytes(), iid.tobytes())
    if key not in _CACHE:
        sc = _build_schedule(uid, iid)
        nc = _build_program(sc)
        _CACHE[key] = (sc, nc)
    sc, nc = _CACHE[key]

    wstack, misc, B = _prep_weights(inputs)
    wbf = wstack.astype(BF)
    miscbf = misc.astype(BF)
    in_maps = []
    for k in range(NCORES):
        hs = _core_hs(inputs, sc, k)
        blob = _core_blob(sc, k, wbf, miscbf, B, hs)
        in_maps.append({"blob": blob, "vthalf": hs})

    res = run_bass_kernel_spmd(nc, in_maps, list(range(NCORES)), trace=TRACE)
    LAST_EXEC_NS = res.exec_time_ns

    out = np.zeros((sc.nev, 2), np.float32)
    ne, NCH = sc.ne, sc.NCH
    for k in range(NCORES):
        arr = res.results[k]["outg"]
        lflat = arr[:, 0:NCH].T.reshape(-1)[:ne]
        sflat = arr[:, NCH:2 * NCH].T.reshape(-1)[:ne]
        mask = sc.gid[k] >= 0
        g = sc.gid[k][mask]
        out[g, 0] = lflat[mask]
        out[g, 1] = sflat[mask]
    return out


# revision 4
# speedup vs baseline: 2.1236x; 1.3318x over previous
"""DeepCoevolve on Trainium2 (Bass/Tile), 8 NeuronCores — v3.

Design notes
------------
1. The reference returns only per-event (loss, score); final embedding
   tables are discarded.  GRU updates are computed only for events whose
   user/item row is read again later ("parents", ~232 of 4096); everything
   else is feed-forward MLP + dot on host-gathered initial embeddings.
2. Only sigmoid/tanh/relu are used on the Scalar engine (one ACT table
   set, warmed during input DMA).  The loss -ln(softplus(d)+1e-10) is a
   degree-4 polynomial in d (|d| < 0.12; fit on [-0.25,0.25], err 3e-8)
   evaluated on DVE over a [128, NCH] transposed dot layout.
3. Scores/dots are computed transposed (events on partitions) via
   lhsT=data matmuls so the final sigmoid/poly run 128-wide.
4. bf16 matmul operands (FWL, 2x rate), f32 PSUM + f32 GRU elementwise.
5. GRU gate biases are folded with K=2 bias-pair matmuls against a 0/1
   selector (keeps one full-width ACT per gate, no per-half bias calls).
6. Inputs packed into 3 DMA triggers: A = weights/sel/bias/idx/parent
   prefill (small, gates step A), C = f32 gather source, B = bf16 hs
   mirror for the bulk MLP.
7. Chain levels (1..4) each do: one fused ap_gather (u+v lanes) from the
   f32 value buffer -> staging, casts to the bf16 mirror, 16 small
   matmuls, 3 ACT + 6 DVE ops.  Bulk MLP chunks are emitted between chain
   steps so the PE works during the ~1.3us gather dispatch latency.
   Only a 16-column MLP slice waits for the last gather.
"""

import numpy as np
import ml_dtypes
from contextlib import ExitStack

E = 128
NCORES = 8
L0 = 512
BF = ml_dtypes.bfloat16

_CACHE = {}
LAST_EXEC_NS = None
TRACE = False

# P(d) ~= -ln(ln(1+e^d)+1e-10), fit on [-0.25, 0.25], max err 3.3e-8
_PC = [0.3665129211512359, -0.7213472868356873, 0.07983400245294202,
       0.004952243233654431, -0.00236161488983429]

WCOLS = 12 * E + 2 * E + 32          # weight cols: 12 GRU blocks, t1a/t1b, t2
BR, BZ, BI, BH = WCOLS, WCOLS + E, WCOLS + 2 * E, WCOLS + 3 * E  # bias pairs
WTOT = WCOLS + 4 * E                 # 2336 bf16 cols


def _r16(x):
    return max(16, (int(x) + 15) // 16 * 16)


class _S:
    pass


# ----------------------------------------------------------------------------
# host-side scheduling
# ----------------------------------------------------------------------------

def _build_schedule(uid, iid):
    uid = np.asarray(uid, np.int64)
    iid = np.asarray(iid, np.int64)
    nev = len(uid)

    lvl = np.zeros(nev, np.int32)
    ispar = np.zeros(nev, bool)
    last_u, last_i = {}, {}
    par = list(range(nev))

    def find(x):
        while par[x] != x:
            par[x] = par[par[x]]
            x = par[x]
        return x

    for e in range(nev):
        l = 0
        for prev in (last_u.get(uid[e]), last_i.get(iid[e])):
            if prev is not None:
                l = max(l, lvl[prev] + 1)
                ispar[prev] = True
                ra, rb = find(e), find(prev)
                if ra != rb:
                    par[ra] = rb
        lvl[e] = l
        last_u[uid[e]] = e
        last_i[iid[e]] = e
    nlev = int(lvl.max()) + 1

    comps = {}
    for e in range(nev):
        comps.setdefault(find(e), []).append(e)
    comp_list = sorted(comps.values(), key=len, reverse=True)
    core_events = [[] for _ in range(NCORES)]
    tot = [0] * NCORES
    for c in comp_list:
        k = min(range(NCORES), key=lambda i: tot[i])
        core_events[k].extend(c)
        tot[k] += len(c)

    queues = [[[] for _ in range(nlev)] for _ in range(NCORES)]
    for k in range(NCORES):
        for e in sorted(core_events[k]):
            queues[k][lvl[e]].append(e)
        for l in range(nlev):
            queues[k][l].sort(key=lambda e: (not ispar[e], e))

    assert max(len(queues[k][0]) for k in range(NCORES)) <= L0
    L = [L0] + [_r16(max(len(queues[k][l]) for k in range(NCORES)))
                for l in range(1, nlev)]
    BP = []
    for l in range(nlev):
        bp = max(sum(1 for e in queues[k][l] if ispar[e])
                 for k in range(NCORES))
        BP.append((bp + 3) // 4 * 4 if bp else 0)
    off = [0] * nlev
    for l in range(1, nlev):
        off[l] = off[l - 1] + L[l - 1]
    ne = off[-1] + L[-1]
    NCH = (ne + 127) // 128
    assert ne - L0 <= 128                    # single chain output column
    SB = {}
    cum = 2 * ne
    for l in range(nlev):
        if BP[l]:
            SB[l] = cum
            cum += 2 * BP[l]
    NV = cum
    assert NV < 32000

    gid = np.full((NCORES, ne), -1, np.int32)
    pre_u = np.full((NCORES, ne), -1, np.int64)
    pre_v = np.full((NCORES, ne), -1, np.int64)
    usrc = np.zeros((NCORES, ne), np.int32)
    vsrc = np.zeros((NCORES, ne), np.int32)
    for k in range(NCORES):
        lsu, lsi = {}, {}
        for l in range(nlev):
            q = queues[k][l]
            assert len(q) <= L[l]
            for j in range(L[l]):
                s = off[l] + j
                if j < len(q):
                    e = q[j]
                    gid[k, s] = e
                    u, i = uid[e], iid[e]
                    if u in lsu:
                        usrc[k, s] = lsu[u]
                    else:
                        usrc[k, s] = s
                        pre_u[k, s] = u
                    if i in lsi:
                        vsrc[k, s] = lsi[i]
                    else:
                        vsrc[k, s] = ne + s
                        pre_v[k, s] = i
                    if ispar[e]:
                        assert j < BP[l]
                        lsu[u] = SB[l] + j
                        lsi[i] = SB[l] + BP[l] + j
                else:
                    usrc[k, s] = s
                    vsrc[k, s] = ne + s

    ic = [0] * nlev
    cols = 0
    for l in range(1, nlev):
        ic[l] = cols
        cols += (2 * L[l]) // 16
    NIC = max(cols, 2)
    assert NIC % 2 == 0
    gidx = np.zeros((NCORES, 16, NIC), np.int16)
    for k in range(NCORES):
        for l in range(1, nlev):
            lanes = np.concatenate(
                [usrc[k, off[l]:off[l] + L[l]],
                 vsrc[k, off[l]:off[l] + L[l]]]).astype(np.int16)
            gidx[k, :, ic[l]:ic[l] + len(lanes) // 16] = \
                lanes.reshape(-1, 16).T

    sc = _S()
    sc.nev, sc.ne, sc.nlev, sc.NCH, sc.NV, sc.NIC = nev, ne, nlev, NCH, NV, NIC
    sc.L, sc.BP, sc.off, sc.SB, sc.ic = L, BP, off, SB, ic
    sc.gid, sc.pre_u, sc.pre_v = gid, pre_u, pre_v
    sc.usrc, sc.vsrc, sc.gidx = usrc, vsrc, gidx
    sc.pure = [(c * 128, 128, c) for c in range(L0 // 128)]
    # sel section layout (misc cols after t3/ones)
    sc.selb = sorted({b for b in BP if b}, reverse=True)
    so = 2
    sc.sel_off = {}
    for b in sc.selb:
        sc.sel_off[b] = so
        so += 2 * b
    sc.NMISC = so
    # blob A byte offsets
    sc.OFF_MISC = 2 * WTOT
    sc.OFF_B = sc.OFF_MISC + ((2 * sc.NMISC + 3) // 4 * 4)
    sc.OFF_IDX = sc.OFF_B + 48
    sc.OFF_PPF = sc.OFF_IDX + 2 * NIC
    assert sc.OFF_PPF % 4 == 0
    sc.BA = sc.OFF_PPF + 4 * BP[0]
    return sc


# ----------------------------------------------------------------------------
# host-side data prep
# ----------------------------------------------------------------------------

def _prep_weights(inp, sc):
    f = np.float32
    uwi, uwh = inp["ugru_wi"].astype(f), inp["ugru_wh"].astype(f)
    iwi, iwh = inp["igru_wi"].astype(f), inp["igru_wh"].astype(f)
    t1w, t2w, t3w = (inp["t1_w"].astype(f), inp["t2_w"].astype(f),
                     inp["t3_w"].astype(f))
    blocks = []
    for g in (0, 1):                                  # r, z gates
        s = slice(g * E, (g + 1) * E)
        blocks += [uwi[s].T, uwh[s].T, iwi[s].T, iwh[s].T]
    s = slice(2 * E, 3 * E)
    blocks += [uwi[s].T, iwi[s].T]                    # inn (applied to x)
    blocks += [uwh[s].T, iwh[s].T]                    # hn  (applied to h)
    blocks += [t1w[:, :E].T, t1w[:, E:].T, t2w.T]
    wstack = np.zeros((E, WTOT), f)
    wstack[:, 0:WCOLS] = np.concatenate(blocks, axis=1)

    ubi, ubh = inp["ugru_bi"].astype(f), inp["ugru_bh"].astype(f)
    ibi, ibh = inp["igru_bi"].astype(f), inp["igru_bh"].astype(f)
    # bias-pair blocks: partitions 0/1 = user/item bias row
    wstack[0, BR:BR + E] = ubi[0:E] + ubh[0:E]
    wstack[1, BR:BR + E] = ibi[0:E] + ibh[0:E]
    wstack[0, BZ:BZ + E] = ubi[E:2 * E] + ubh[E:2 * E]
    wstack[1, BZ:BZ + E] = ibi[E:2 * E] + ibh[E:2 * E]
    wstack[0, BI:BI + E] = ubi[2 * E:]
    wstack[1, BI:BI + E] = ibi[2 * E:]
    wstack[0, BH:BH + E] = ubh[2 * E:]
    wstack[1, BH:BH + E] = ibh[2 * E:]

    misc = np.zeros((E, sc.NMISC), f)
    misc[:32, 0] = t3w[0]
    misc[:, 1] = 1.0
    for b in sc.selb:
        so = sc.sel_off[b]
        misc[0, so:so + b] = 1.0
        misc[1, so + b:so + 2 * b] = 1.0

    B = np.zeros((E, 12), f)
    B[:, 8] = inp["t1_b"].astype(f)
    B[:32, 9] = inp["t2_b"].astype(f)
    B[:, 10] = inp["t3_b"].astype(f)[0]
    return wstack, misc, B


def _core_hs(inp, sc, k):
    ne = sc.ne
    hs = np.zeros((E, 2 * ne), np.float32)
    mu = sc.pre_u[k] >= 0
    if mu.any():
        hs[:, 0:ne][:, mu] = inp["user_emb"][sc.pre_u[k][mu]].T
    mv = sc.pre_v[k] >= 0
    if mv.any():
        hs[:, ne:][:, mv] = inp["item_emb"][sc.pre_v[k][mv]].T
    return hs


def _core_blobs(sc, k, wbf, miscbf, B, hs):
    ne = sc.ne
    b0 = sc.BP[0]
    blobA = np.zeros((E, sc.BA), np.uint8)
    blobA[:, 0:2 * WTOT] = np.ascontiguousarray(wbf).view(np.uint8)
    blobA[:, sc.OFF_MISC:sc.OFF_MISC + 2 * sc.NMISC] = \
        np.ascontiguousarray(miscbf).view(np.uint8)
    blobA[:, sc.OFF_B:sc.OFF_B + 48] = np.ascontiguousarray(B).view(np.uint8)
    idx = np.tile(sc.gidx[k], (8, 1))
    blobA[:, sc.OFF_IDX:sc.OFF_IDX + 2 * sc.NIC] = \
        np.ascontiguousarray(idx).view(np.uint8)
    ppf = np.concatenate([hs[:, 0:b0], hs[:, ne:ne + b0]], axis=1).astype(BF)
    blobA[:, sc.OFF_PPF:] = np.ascontiguousarray(ppf).view(np.uint8)
    hsb = np.ascontiguousarray(hs.astype(BF))
    return blobA, hsb.view(np.uint8)


# ----------------------------------------------------------------------------
# numpy model (host validation)
# ----------------------------------------------------------------------------

def _numpy_model(inp, sc):
    wstack, misc, B = _prep_weights(inp, sc)
    ne, nlev = sc.ne, sc.nlev
    out = np.zeros((sc.nev, 2), np.float32)

    def blk(i):
        return wstack[:, i * E:(i + 1) * E]

    def sig(x):
        return 1.0 / (1.0 + np.exp(-x))

    for k in range(NCORES):
        hs = _core_hs(inp, sc, k)
        vt = np.zeros((E, sc.NV), np.float32)
        vt[:, 0:2 * ne] = hs
        for l in range(nlev):
            o, Ll = sc.off[l], sc.L[l]
            if l > 0:
                hs[:, o:o + Ll] = vt[:, sc.usrc[k, o:o + Ll]]
                hs[:, ne + o:ne + o + Ll] = vt[:, sc.vsrc[k, o:o + Ll]]
            b = sc.BP[l]
            if not b:
                continue
            ug = hs[:, o:o + b]
            vg = hs[:, ne + o:ne + o + b]
            bru = wstack[0, BR:BR + E][:, None]
            bri = wstack[1, BR:BR + E][:, None]
            bzu = wstack[0, BZ:BZ + E][:, None]
            bzi = wstack[1, BZ:BZ + E][:, None]
            biu = wstack[0, BI:BI + E][:, None]
            bii = wstack[1, BI:BI + E][:, None]
            bhu = wstack[0, BH:BH + E][:, None]
            bhi = wstack[1, BH:BH + E][:, None]
            r = sig(blk(0).T @ vg + blk(1).T @ ug + bru)
            z = sig(blk(4).T @ vg + blk(5).T @ ug + bzu)
            n = np.tanh(blk(8).T @ vg + biu + r * (blk(10).T @ ug + bhu))
            hu = n + z * (ug - n)
            r2 = sig(blk(2).T @ ug + blk(3).T @ vg + bri)
            z2 = sig(blk(6).T @ ug + blk(7).T @ vg + bzi)
            n2 = np.tanh(blk(9).T @ ug + bii + r2 * (blk(11).T @ vg + bhi))
            hv = n2 + z2 * (vg - n2)
            sb = sc.SB[l]
            vt[:, sb:sb + b] = hu
            vt[:, sb + b:sb + 2 * b] = hv
        hsu, hsv = hs[:, 0:ne], hs[:, ne:]
        t1a = wstack[:, 12 * E:13 * E]
        t1b = wstack[:, 13 * E:14 * E]
        t2 = wstack[:, 14 * E:14 * E + 32]
        h1 = np.maximum(t1a.T @ hsu + t1b.T @ hsv + B[:, 8:9], 0.0)
        h2 = np.maximum(t2.T @ h1 + B[:32, 9:10], 0.0)
        sco = sig(misc[:32, 0] @ h2 + B[0, 10])
        d = (hsu * hsv).sum(axis=0)
        p = np.full_like(d, _PC[4])
        for c in _PC[3::-1]:
            p = p * d + c
        mask = sc.gid[k] >= 0
        g = sc.gid[k][mask]
        out[g, 0] = p[mask]
        out[g, 1] = sco[mask]
    return out


# ----------------------------------------------------------------------------
# device program
# ----------------------------------------------------------------------------

def _build_program(sc):
    import concourse.bass as bass  # noqa: F401
    import concourse.tile as tile
    from concourse import bacc, mybir
    from concourse.tile_rust import add_dep_helper

    f32 = mybir.dt.float32
    bf16 = mybir.dt.bfloat16
    i16 = mybir.dt.int16
    u8 = mybir.dt.uint8
    AF = mybir.ActivationFunctionType
    OP = mybir.AluOpType
    ne, NV, NCH, nlev = sc.ne, sc.NV, sc.NCH, sc.nlev

    nc = bacc.Bacc("TRN2", target_bir_lowering=False, debug=False)
    d_A = nc.dram_tensor("blobA", [E, sc.BA], u8, kind="ExternalInput").ap()
    d_B = nc.dram_tensor("hsbu8", [E, 4 * ne], u8, kind="ExternalInput").ap()
    d_C = nc.dram_tensor("vthalf", [E, 2 * ne], f32,
                         kind="ExternalInput").ap()
    d_out = nc.dram_tensor("outg", [128, 2 * NCH], f32,
                           kind="ExternalOutput").ap()

    with tile.TileContext(nc) as tc, ExitStack() as ctx:
        const = ctx.enter_context(tc.tile_pool(name="const", bufs=1))
        psA = ctx.enter_context(tc.tile_pool(name="psA", bufs=2, space="PSUM"))
        psB = ctx.enter_context(tc.tile_pool(name="psB", bufs=1, space="PSUM"))
        acc = ctx.enter_context(tc.tile_pool(name="acc", bufs=1, space="PSUM"))
        work = ctx.enter_context(tc.tile_pool(name="work", bufs=2))

        blobA = const.tile([E, sc.BA], u8)
        hsbt = const.tile([E, 4 * ne], u8)
        vt = const.tile([E, NV], f32)
        dmaA = nc.sync.dma_start(blobA[:], d_A[:])
        dmaC = nc.sync.dma_start(vt[:, 0:2 * ne], d_C[:])
        dmaB = nc.sync.dma_start(hsbt[:], d_B[:])

        # warmups: ACT table set + GPSIMD gather library (run during DMA)
        wtab = const.tile([E, 2], f32)
        nc.vector.memset(wtab[:], 0.0)
        nc.scalar.activation(wtab[:, 1:2], wtab[:, 0:1], AF.Sigmoid, bias=0.0)
        warm = const.tile([E, 16], f32)
        nc.vector.memset(warm[:], 0.0)
        warmi = const.tile([E, 2], i16)
        nc.vector.memset(warmi[:].bitcast(f32), 0.0)
        warmo = const.tile([E, 16], f32)
        nc.gpsimd.ap_gather(warmo[:], warm[:], warmi[:, 0:1],
                            channels=E, num_elems=16, d=1, num_idxs=16)

        wsb = blobA[:, 0:2 * WTOT].bitcast(bf16)
        miscb = blobA[:, sc.OFF_MISC:sc.OFF_MISC + 2 * sc.NMISC].bitcast(bf16)
        bias = blobA[:, sc.OFF_B:sc.OFF_B + 48].bitcast(f32)
        idxt = blobA[:, sc.OFF_IDX:sc.OFF_IDX + 2 * sc.NIC].bitcast(i16)
        ppf = blobA[:, sc.OFF_PPF:].bitcast(bf16)
        hsb = hsbt[:].bitcast(bf16)

        SW = max(max(sc.L[1:], default=16), sc.BP[0])
        stag = const.tile([E, 2 * SW], f32)
        outsb = const.tile([128, 2 * NCH], f32)
        pscore = acc.tile([128, NCH], f32, tag="pscore")
        pdot = acc.tile([128, NCH], f32, tag="pdot")

        def mmw(out_ap, col, ncols, rhs, start, stop):
            nc.tensor.matmul(out_ap, lhsT=wsb[:, col:col + ncols], rhs=rhs,
                             start=start, stop=stop, skip_group_check=True)

        def gru_step(l, sw, ug, vg):
            b = sc.BP[l]
            selb = miscb[0:2, sc.sel_off[b]:sc.sel_off[b] + 2 * b]

            def gate(pt, bcol, plan):
                nc.tensor.matmul(pt[:, 0:2 * b], lhsT=wsb[0:2, bcol:bcol + E],
                                 rhs=selb, start=True, stop=False,
                                 skip_group_check=True)
                for i, (wc, rh, half) in enumerate(plan):
                    mmw(pt[:, half * b:(half + 1) * b], wc * E, E, rh,
                        False, i == len(plan) - 1)

            pz = psA.tile([E, 2 * b], f32, tag="pz")
            pr = psA.tile([E, 2 * b], f32, tag="pr")
            phn = psB.tile([E, 2 * b], f32, tag="phn")
            pinn = psB.tile([E, 2 * b], f32, tag="pinn")
            gate(pz, BZ, [(4, vg, 0), (5, ug, 0), (6, ug, 1), (7, vg, 1)])
            gate(pr, BR, [(0, vg, 0), (1, ug, 0), (2, ug, 1), (3, vg, 1)])
            gate(phn, BH, [(10, ug, 0), (11, vg, 1)])
            gate(pinn, BI, [(8, vg, 0), (9, ug, 1)])

            z = work.tile([E, 2 * b], f32, tag="z")
            r = work.tile([E, 2 * b], f32, tag="r")
            zh = work.tile([E, 2 * b], f32, tag="zh")
            m = work.tile([E, 2 * b], f32, tag="m")
            nf = work.tile([E, 2 * b], f32, tag="nf")
            tmp = work.tile([E, 2 * b], f32, tag="tmp")
            nc.scalar.activation(z[:], pz[:], AF.Sigmoid, bias=0.0)
            nc.scalar.activation(r[:], pr[:], AF.Sigmoid, bias=0.0)
            hcat3 = stag[:, 0:2 * sw].rearrange(
                "p (t x) -> p t x", t=2)[:, :, 0:b]
            z3 = z[:].rearrange("p (t x) -> p t x", t=2)
            zh3 = zh[:].rearrange("p (t x) -> p t x", t=2)
            nc.vector.tensor_tensor(out=zh3, in0=z3, in1=hcat3, op=OP.mult)
            nc.vector.tensor_scalar(out=m[:], in0=z[:], scalar1=-1.0,
                                    scalar2=1.0, op0=OP.mult, op1=OP.add)
            nc.vector.tensor_tensor(out=tmp[:], in0=r[:], in1=phn[:],
                                    op=OP.mult)
            nc.vector.tensor_tensor(out=tmp[:], in0=tmp[:], in1=pinn[:],
                                    op=OP.add)
            nc.scalar.activation(nf[:], tmp[:], AF.Tanh, bias=0.0)
            nc.vector.tensor_tensor(out=tmp[:], in0=nf[:], in1=m[:],
                                    op=OP.mult)
            sb = sc.SB[l]
            return nc.vector.tensor_tensor(out=vt[:, sb:sb + 2 * b],
                                           in0=tmp[:], in1=zh[:], op=OP.add)

        def chunk(c0, cb, cc, ro):
            ub = hsb[:, c0:c0 + cb]
            vb = hsb[:, ne + c0:ne + c0 + cb]
            h1p = psA.tile([E, cb], f32, tag="pz")
            mmw(h1p[:], 12 * E, E, ub, True, False)
            mmw(h1p[:], 13 * E, E, vb, False, True)
            h1 = work.tile([E, cb], bf16, tag="h1")
            nc.scalar.activation(h1[:], h1p[:], AF.Relu, bias=bias[:, 8:9])
            h2p = psA.tile([32, cb], f32, tag="pr")
            mmw(h2p[:], 14 * E, 32, h1[:], True, True)
            h2 = work.tile([32, cb], bf16, tag="h2")
            nc.scalar.activation(h2[:], h2p[:], AF.Relu,
                                 bias=bias[0:32, 9:10])
            nc.tensor.matmul(pscore[ro:ro + cb, cc:cc + 1], lhsT=h2[:],
                             rhs=miscb[0:32, 0:1], start=True, stop=True,
                             skip_group_check=True)
            uvm = work.tile([E, cb], bf16, tag="uvm")
            nc.vector.tensor_tensor(out=uvm[:], in0=ub, in1=vb, op=OP.mult)
            nc.tensor.matmul(pdot[ro:ro + cb, cc:cc + 1], lhsT=uvm[:],
                             rhs=miscb[:, 1:2], start=True, stop=True,
                             skip_group_check=True)

        # --- step A: level-0 parents (host-prefilled inputs) ---
        b0 = sc.BP[0]
        anchor = None
        if b0:
            nc.vector.tensor_copy(out=stag[:, 0:2 * b0], in_=ppf[:, 0:2 * b0])
            anchor = gru_step(0, b0, ppf[:, 0:b0], ppf[:, b0:2 * b0])

        pure = list(sc.pure)
        pi = 0
        last_off = sc.off[nlev - 1] if nlev > 1 else None

        # --- chain levels ---
        for l in range(1, nlev):
            if l == nlev - 1 and last_off > L0:
                # chain-region MLP for levels 1..nlev-2 (ready before the
                # last gather) so only a small slice waits on it
                chunk(L0, last_off - L0, NCH - 1, 0)
            Ll = sc.L[l]
            o = sc.off[l]
            g = nc.gpsimd.ap_gather(
                stag[:, 0:2 * Ll], vt[:],
                idxt[:, sc.ic[l]:sc.ic[l] + 2 * Ll // 16],
                channels=E, num_elems=NV, d=1, num_idxs=2 * Ll)
            if anchor is not None:
                add_dep_helper(g.ins, anchor.ins,
                               reason="gather reads prev writeback")
            add_dep_helper(g.ins, dmaC.ins, reason="gather reads vt dma")
            add_dep_helper(g.ins, dmaA.ins, reason="gather reads idx dma")
            c1 = nc.vector.tensor_copy(out=hsb[:, o:o + Ll],
                                       in_=stag[:, 0:Ll])
            c2 = nc.vector.tensor_copy(out=hsb[:, ne + o:ne + o + Ll],
                                       in_=stag[:, Ll:2 * Ll])
            add_dep_helper(c1.ins, g.ins, reason="cast reads gather out")
            add_dep_helper(c2.ins, g.ins, reason="cast reads gather out")
            if pi < len(pure):
                chunk(*pure[pi], 0)
                pi += 1
            if sc.BP[l]:
                anchor = gru_step(l, Ll, hsb[:, o:o + sc.BP[l]],
                                  hsb[:, ne + o:ne + o + sc.BP[l]])
            else:
                anchor = c2
        while pi < len(pure):
            chunk(*pure[pi], 0)
            pi += 1
        if nlev > 1:
            # the only slots that wait for the last gather
            chunk(last_off, ne - last_off, NCH - 1, last_off - L0)

        # --- scores + polynomial losses, 128-wide ---
        nc.scalar.activation(outsb[:, NCH:2 * NCH], pscore[:], AF.Sigmoid,
                             bias=bias[:, 10:11])
        pt = const.tile([128, NCH], f32)
        nc.vector.tensor_scalar(out=pt[:], in0=pdot[:], scalar1=_PC[4],
                                scalar2=_PC[3], op0=OP.mult, op1=OP.add)
        for k in range(2, -1, -1):
            nc.vector.tensor_tensor(out=pt[:], in0=pt[:], in1=pdot[:],
                                    op=OP.mult)
            dst = outsb[:, 0:NCH] if k == 0 else pt[:]
            nc.vector.tensor_scalar(out=dst, in0=pt[:], scalar1=_PC[k],
                                    scalar2=None, op0=OP.add)
        nc.sync.dma_start(d_out[:], outsb[:])

    nc.compile()
    return nc


# ----------------------------------------------------------------------------
# entry point
# ----------------------------------------------------------------------------

def kernel(**inputs):
    global LAST_EXEC_NS
    from concourse.bass_utils import run_bass_kernel_spmd

    uid = np.asarray(inputs["user_ids"])
    iid = np.asarray(inputs["item_ids"])
    key = (uid.tobytes(), iid.tobytes())
    if key not in _CACHE:
        sc = _build_schedule(uid, iid)
        nc = _build_program(sc)
        _CACHE[key] = (sc, nc)
    sc, nc = _CACHE[key]

    wstack, misc, B = _prep_weights(inputs, sc)
    wbf = wstack.astype(BF)
    miscbf = misc.astype(BF)
    in_maps = []
    for k in range(NCORES):
        hs = _core_hs(inputs, sc, k)
        blobA, hsbu8 = _core_blobs(sc, k, wbf, miscbf, B, hs)
        in_maps.append({"blobA": blobA, "hsbu8": hsbu8, "vthalf": hs})

    res = run_bass_kernel_spmd(nc, in_maps, list(range(NCORES)), trace=TRACE)
    LAST_EXEC_NS = res.exec_time_ns

    out = np.zeros((sc.nev, 2), np.float32)
    ne, NCH = sc.ne, sc.NCH
    for k in range(NCORES):
        arr = res.results[k]["outg"]
        lflat = arr[:, 0:NCH].T.reshape(-1)[:ne]
        sflat = arr[:, NCH:2 * NCH].T.reshape(-1)[:ne]
        mask = sc.gid[k] >= 0
        g = sc.gid[k][mask]
        out[g, 0] = lflat[mask]
        out[g, 1] = sflat[mask]
    return out
